# revision 59
# baseline (speedup 1.0000x reference)
"""CARAFE (content-aware upsampling) Trainium2 kernel — v2.

Full inputs -> shard over 8 NeuronCores (batch x image-half) -> bass/Tile
kernel per core -> gather full output.

Reference semantics:
  comp = conv1x1(x, w_comp) + b_comp                    [n,64,64,64]
  mask = conv3x3(comp, w_enc, pad=1) + b_enc            [n,100,64,64]
  m    = softmax over 25 of pixel_shuffle(mask, 2)      [n,25,128,128]
  out[n,c,i,j] = sum_k m[n,k,i,j] * xpad[n,c,i//2+p, j//2+q],  k=5p+q

v2 design:
  - bf16 operands on-chip (fp32 PSUM accumulate, fp32 output)
  - mask conv channel-major (weights stationary), exp+bias fused in evac,
    PE transpose to pixel-major, DVE Z-reduce/recip/normalize
  - jl->w' stagger via 5 constant 0/1 shift-matrix matmuls (paired over
    2-block superblocks)
  - band shear via DRAM flat-address scatter (SBUF-side shears silently
    fail on HW) into reused, once-zeroed DRAM buffers; read back as
    [64, 4*HB2] with 40-col guards absorbing all clipped/invalid cells
  - x loaded once column-major [w', slabrow, ch] for carafe lhsT slices
"""
import numpy as np
import sys
from contextlib import ExitStack

sys.path.insert(0, "/opt/trn_rl_repo")

# ---------------- problem constants (hardcoded per spec) ----------------
N_B, C, H, W = 4, 256, 64, 64
CC = 64            # compressed channels
K5 = 5             # carafe kernel
S = 2              # scale
CM = K5 * K5 * S * S   # 100 mask channels
NCORES = 8
RH = H // 2        # 32 low-res rows per core
SLAB = RH + 4      # 36 x-rows per core (h0-2 .. h0+33)
NBLK = RH // 2     # 16 h-pair blocks
NSUP = NBLK // 2   # 8 superblocks (2 h-pair blocks each)
HO, WO = 2 * RH, 2 * W   # 64 x 128 output shard
GUARD = 40
HB2 = W * 20 + 2 * GUARD   # 1360 band cols per (rr)
SBW = 4 * HB2      # 5440 band cols per superblock (tt, rr)
COMP_W = W + 2     # 66 comp cols (1 zero col each side)
COMP_R = RH + 2    # 34 comp rows


def _build_program():
    import concourse.bass as bass
    import concourse.tile as tile
    from concourse import bacc, mybir
    from concourse.ap import AP
    AF = mybir.ActivationFunctionType

    def pstep(t):
        return t[:].ap[0][0]

    f32 = mybir.dt.float32
    bf16 = mybir.dt.bfloat16

    nc = bacc.Bacc("TRN2", target_bir_lowering=False, debug=False,
                   num_devices=NCORES)

    # ---------------- DRAM parameters ----------------
    xs = nc.dram_tensor("xs", [C, SLAB * W], bf16, kind="ExternalInput")
    xb = nc.dram_tensor("xb", [W, SLAB * C], bf16, kind="ExternalInput")
    wcT = nc.dram_tensor("wcT", [C, CC], bf16, kind="ExternalInput")
    bc = nc.dram_tensor("bc", [CC, 1], f32, kind="ExternalInput")
    weP = nc.dram_tensor("weP", [3, 2 * CC, CM], bf16, kind="ExternalInput")
    weS = nc.dram_tensor("weS", [3, CC, CM], bf16, kind="ExternalInput")
    benc = nc.dram_tensor("benc", [CM, 1], f32, kind="ExternalInput")
    shf = nc.dram_tensor("shf", [128, K5 * 128], bf16, kind="ExternalInput")
    ident = nc.dram_tensor("ident", [CM, CM], bf16, kind="ExternalInput")
    bndz = [nc.dram_tensor(f"bndz{i}", [W, 2 * SBW], bf16) for i in range(2)]
    out = nc.dram_tensor("out", [C, HO, WO], f32, kind="ExternalOutput")

    with tile.TileContext(nc) as tc:
        with ExitStack() as ctx:
            cpool = ctx.enter_context(tc.tile_pool(name="const", bufs=1))
            xpool = ctx.enter_context(tc.tile_pool(name="xdata", bufs=1))
            work = ctx.enter_context(tc.tile_pool(name="work", bufs=3))
            bpool = ctx.enter_context(tc.tile_pool(name="bandp", bufs=2))
            opool = ctx.enter_context(tc.tile_pool(name="oevac", bufs=3))
            ps_c_p = ctx.enter_context(tc.tile_pool(name="psc", bufs=1, space="PSUM"))
            ps_cm_p = ctx.enter_context(tc.tile_pool(name="pscm", bufs=2, space="PSUM"))
            ps_m_p = ctx.enter_context(tc.tile_pool(name="psm", bufs=2, space="PSUM"))
            ps_r_p = ctx.enter_context(tc.tile_pool(name="psr", bufs=1, space="PSUM"))
            ps_o_p = ctx.enter_context(tc.tile_pool(name="pso", bufs=2, space="PSUM"))

            # ---------------- load constants ----------------
            t_wc = [cpool.tile([128, CC], bf16, tag=f"wc{k}", name=f"wc{k}")
                    for k in range(2)]
            for k in range(2):
                nc.sync.dma_start(t_wc[k][:], wcT.ap()[128 * k:128 * (k + 1), :])
            t_bc = cpool.tile([CC, 1], f32, tag="bc", name="bc")
            nc.sync.dma_start(t_bc[:], bc.ap())
            t_weP = cpool.tile([2 * CC, 3 * CM], bf16, tag="weP", name="weP")
            src_wp = AP(weP.ap().tensor, 0,
                        [[CM, 2 * CC], [2 * CC * CM, 3], [1, CM]])
            nc.sync.dma_start(t_weP[:], src_wp)
            t_weS = cpool.tile([CC, 3 * CM], bf16, tag="weS", name="weS")
            src_ws = AP(weS.ap().tensor, 0,
                        [[CM, CC], [CC * CM, 3], [1, CM]])
            nc.sync.dma_start(t_weS[:], src_ws)
            t_benc = cpool.tile([CM, 1], f32, tag="benc", name="benc")
            nc.sync.dma_start(t_benc[:], benc.ap())
            t_id = cpool.tile([CM, CM], bf16, tag="ident", name="ident")
            nc.sync.dma_start(t_id[:], ident.ap())
            t_shf = cpool.tile([128, K5 * 128], bf16, tag="shf", name="shf")
            nc.sync.dma_start(t_shf[:], shf.ap())

            # ---------------- load x ----------------
            t_xs = [xpool.tile([128, SLAB * W], bf16, tag=f"xs{k}", name=f"xs{k}")
                    for k in range(2)]
            for k in range(2):
                nc.sync.dma_start(t_xs[k][:], xs.ap()[128 * k:128 * (k + 1), :])
            t_xb = xpool.tile([W, SLAB * C], bf16, tag="xb", name="xb")
            xbstep = pstep(t_xb)

            # ---------------- zero the DRAM band buffers (once) -------------
            # bndz[0] is needed by scatter(0) early; bndz[1]'s zero-write is
            # deferred below to keep it off the startup-critical DMA rings
            t_bz = bpool.tile([W, 2 * SBW], bf16, tag="band", name="bz")
            nc.gpsimd.memset(t_bz[:], 0.0)
            nc.scalar.dma_start(bndz[0].ap(), t_bz[:])

            # ------------- per superblock (2 h-pair blocks) -----------------
            # Software-pipelined: mask_phase(u+1) is emitted between
            # scatter(u) and carafe(u) so the PE has work while the band
            # round-trips through DRAM.
            def phase_A(u, t_ms):
                """Mask math for superblock u: comp conv, 3x3 conv, exp,
                transpose, Z, reciprocal, normalize. Returns t_ms."""
                # comp = 1x1 conv + bias for this superblock's 6 rows
                # (comp rows 4u..4u+5 = slab rows 4u+1..4u+6); half 1 of the
                # partition dim holds comp shifted left one col for tap pairs
                t_comp = work.tile([2 * CC, 6 * COMP_W], bf16, tag="comp",
                                   name="comp")
                compv = t_comp[:].rearrange("p (r w) -> p r w", w=COMP_W)
                nc.vector.memset(compv[0:CC, :, 0:1], 0.0)
                nc.vector.memset(compv[0:CC, :, COMP_W - 1:COMP_W], 0.0)
                nc.vector.memset(compv[CC:2 * CC, :, COMP_W - 2:COMP_W], 0.0)
                cstep = pstep(t_comp)
                ps_c = ps_c_p.tile([CC, 6 * W], f32, tag="ps_c", name="ps_c")
                for k in range(2):
                    rhs = AP(t_xs[k][:].tensor, (4 * u + 1) * W,
                             [[pstep(t_xs[k]), 128], [1, 6 * W]])
                    nc.tensor.matmul(ps_c[:], t_wc[k][:], rhs,
                                     start=(k == 0), stop=(k == 1))
                psv = ps_c[:].rearrange("p (r w) -> p r w", w=W)
                nc.scalar.activation(compv[0:CC, :, 1:1 + W], psv,
                                     func=AF.Identity, bias=t_bc[:])
                nc.scalar.activation(compv[CC:2 * CC, :, 0:W], psv,
                                     func=AF.Identity, bias=t_bc[:])

                ps_m = ps_m_p.tile([128, 2 * CM], bf16, tag="ps_m", name="ps_m")
                mstep = ps_m[:].ap[0][0]
                t_z = work.tile([128, 8], f32, tag="z", name="z")
                zst = pstep(t_z)
                # mask conv, channel-major: ps_cm [100, 128 pix=(rr,w')]
                # tap pairs (dx=0,1) over 128 partitions + single dx=2;
                # the two tt interleave so consecutive matmuls hit
                # different PSUM tiles (accumulation chains pipeline)
                ps_cm = [ps_cm_p.tile([CM, 128], f32, tag="ps_cm",
                                      name=f"ps_cm{tt}") for tt in range(2)]
                for dy in range(3):
                    for tt in range(2):
                        rhsP = AP(t_comp[:].tensor, (2 * tt + dy) * COMP_W,
                                  [[cstep, 2 * CC], [COMP_W, 2], [1, W]])
                        nc.tensor.matmul(ps_cm[tt][:],
                                         t_weP[:, dy * CM:(dy + 1) * CM],
                                         rhsP, start=(dy == 0), stop=False)
                    for tt in range(2):
                        rhsS = AP(t_comp[:].tensor,
                                  (2 * tt + dy) * COMP_W + 2,
                                  [[cstep, CC], [COMP_W, 2], [1, W]])
                        nc.tensor.matmul(ps_cm[tt][:],
                                         t_weS[:, dy * CM:(dy + 1) * CM],
                                         rhsS, start=False, stop=(dy == 2))
                t_em = [None, None]
                for tt in range(2):
                    # exp(mask + b_enc) -> bf16, channel-major
                    t_em[tt] = work.tile([CM, 128], bf16, tag="em", name="em")
                    nc.scalar.activation(t_em[tt][:], ps_cm[tt][:],
                                         func=AF.Exp, bias=t_benc[:])
                for tt in range(2):
                    # transpose -> pixel-major psum segment [128, 100]
                    nc.tensor.transpose(ps_m[:, tt * CM:(tt + 1) * CM],
                                        t_em[tt][:], t_id[:])
                for tt in range(2):
                    # Z = sum over (qd, p) per (a, b)
                    in_z = AP(ps_m[:].tensor, tt * CM,
                              [[mstep, 128], [10, 2], [1, 2], [20, 5], [2, 5]])
                    oz = AP(t_z[:].tensor, 4 * tt,
                            [[zst, 128], [2, 2], [1, 2]])
                    nc.vector.tensor_reduce(oz, in_z,
                                            axis=mybir.AxisListType.XY,
                                            op=mybir.AluOpType.add)
                t_rz = work.tile([128, 8], bf16, tag="rz", name="rz")
                with nc.allow_low_precision(reason="1/Z well-conditioned"):
                    nc.vector.reciprocal(t_rz[:], t_z[:])
                zstep = pstep(t_rz)

                # normalize: t_ms = exp * rz; cols (qd, T, a, p, b)
                # where T = 2*(u%2) + tt indexes the 4 blocks of the pair
                sstep = pstep(t_ms)
                for tt in range(2):
                    T = 2 * (u % 2) + tt
                    for a in range(2):
                        for b in range(2):
                            o = AP(t_ms[:].tensor, T * 20 + 10 * a + b,
                                   [[sstep, 128], [80, 5], [2, 5]])
                            i0 = AP(ps_m[:].tensor, tt * CM + 10 * a + b,
                                    [[mstep, 128], [20, 5], [2, 5]])
                            i1 = AP(t_rz[:].tensor, 4 * tt + 2 * a + b,
                                    [[zstep, 128], [0, 5], [0, 5]])
                            nc.vector.tensor_mul(o, i0, i1)

            def phase_B2(v, t_ms):
                """Stagger matmuls, evac, DRAM shear scatter, band read for
                a PAIR of superblocks (2v, 2v+1); T = 2*sp + tt in 0..4."""
                sstep = pstep(t_ms)
                # stagger via 5 shift-matmuls over all 4 blocks at once
                ps_r = ps_r_p.tile([128, 4 * CM], f32, tag="ps_r", name="ps_r")
                rpst = ps_r[:].ap[0][0]
                for qd in range(K5):
                    nc.tensor.matmul(ps_r[:, qd * 80:(qd + 1) * 80],
                                     t_shf[:, qd * 128:(qd + 1) * 128],
                                     t_ms[:, qd * 80:(qd + 1) * 80],
                                     start=True, stop=True)
                # evac halves -> t_rp2 [64, (T, rr, qd*20+apb)]
                t_rp2 = work.tile([W, 8 * CM], bf16, tag="rp2", name="rp2")
                rstep = pstep(t_rp2)
                for rr in range(2):
                    o = AP(t_rp2[:].tensor, rr * CM,
                           [[rstep, W], [20, 5], [2 * CM, 4], [1, 20]])
                    i = AP(ps_r[:].tensor, 64 * rr * rpst,
                           [[rpst, W], [80, 5], [20, 4], [1, 20]])
                    if rr == 0:
                        nc.vector.tensor_copy(o, i)
                    else:
                        nc.scalar.activation(o, i, func=AF.Copy)

                # sheared scatter SBUF -> DRAM (flat), one DMA per (sp, tt)
                for sp in range(2):
                    for tt in range(2):
                        dst = AP(bndz[v % 2].ap().tensor,
                                 sp * SBW + tt * 2 * HB2,
                                 [[2 * SBW + 20, W], [HB2, 2], [1, CM]])
                        src = AP(t_rp2[:].tensor, (2 * sp + tt) * 2 * CM,
                                 [[rstep, W], [CM, 2], [1, CM]])
                        with nc.allow_non_contiguous_dma(reason="band shear"):
                            nc.sync.dma_start(dst, src)
                # read back the full pair band
                bnd = bpool.tile([W, 2 * SBW], bf16, tag="band", name="band")
                nc.scalar.dma_start(bnd[:], bndz[v % 2].ap())
                return bnd

            def carafe_phase(u, bnd):
                bstep = pstep(bnd)
                t_o = [opool.tile([128, 1024], f32, tag=f"osb{ct}",
                                  name=f"osb{ct}") for ct in range(2)]
                for tt in range(2):
                    t = 2 * u + tt
                    for ct in range(2):
                        pso = ps_o_p.tile([128, 512], f32, tag="ps_o",
                                          name="ps_o")
                        for rr in range(2):
                            for p in range(K5):
                                lhsT = AP(t_xb[:].tensor,
                                          (2 * t + p + rr) * C + ct * 128,
                                          [[xbstep, W], [1, 128]])
                                rhs = AP(bnd[:].tensor,
                                         (u % 2) * SBW + tt * 2 * HB2
                                         + rr * HB2 + GUARD + 2 * p,
                                         [[bstep, W], [10, 2], [20, W], [1, 2]])
                                nc.tensor.matmul(
                                    pso[:, rr * 256:(rr + 1) * 256],
                                    lhsT, rhs, start=(p == 0),
                                    stop=(p == K5 - 1))
                        dst = t_o[ct][:, tt * 512:(tt + 1) * 512]
                        if ct == 0:
                            nc.vector.tensor_copy(dst, pso[:])
                        else:
                            nc.scalar.activation(dst, pso[:], func=AF.Copy)
                for ct in range(2):
                    dsto = AP(out.ap().tensor,
                              ct * 128 * HO * WO + 8 * u * WO,
                              [[HO * WO, 128], [1, 1024]])
                    nc.sync.dma_start(dsto, t_o[ct][:])

            # 3-stage software pipeline over superblock PAIRS v:
            # PE stream per iteration: stagger(v) -> mask math of upcoming
            # superblocks -> two carafe phases of pair v-1.
            def pairtile():
                return work.tile([128, 4 * CM], bf16, tag="ms", name="ms")

            NPAIR = NSUP // 2
            pair = {0: pairtile()}
            phase_A(0, pair[0])
            nc.scalar.dma_start(bndz[1].ap(), t_bz[:])
            phase_A(1, pair[0])
            pair[1] = pairtile()
            phase_A(2, pair[1])
            bnds2 = {}
            for v in range(NPAIR):
                bnds2[v] = phase_B2(v, pair.pop(v))
                if v == 0:
                    # xb is first needed by carafe(0); defer its load behind
                    # the startup-critical xs/zero/scatter/read DMAs
                    nc.sync.dma_start(t_xb[:], xb.ap())
                if 2 * v + 3 < NSUP:
                    phase_A(2 * v + 3, pair[v + 1])
                if v >= 1:
                    carafe_phase(2 * v - 2, bnds2[v - 1])
                if 2 * v + 4 < NSUP:
                    pair[v + 2] = pairtile()
                    phase_A(2 * v + 4, pair[v + 2])
                if v >= 1:
                    carafe_phase(2 * v - 1, bnds2.pop(v - 1))
            carafe_phase(NSUP - 2, bnds2[NPAIR - 1])
            carafe_phase(NSUP - 1, bnds2.pop(NPAIR - 1))

    nc.compile()
    return nc


_CACHE = {}


def _get_program():
    if "nc" not in _CACHE:
        _CACHE["nc"] = _build_program()
    return _CACHE["nc"]


def host_prep(x, w_comp, b_comp, w_enc, b_enc):
    """Build per-core input maps."""
    from ml_dtypes import bfloat16
    x = np.asarray(x, dtype=np.float32)
    wcT = np.ascontiguousarray(
        np.asarray(w_comp, np.float32).reshape(CC, C).T).astype(bfloat16)
    bc = np.asarray(b_comp, np.float32).reshape(CC, 1)
    # weT2[k, cin, qd*20 + a*10 + p*2 + b] = w_enc[20p+16-4qd+2a+b, cin, dy, dx]
    we = np.asarray(w_enc, np.float32).reshape(CM, CC, 9)
    perm = np.empty(CM, np.int64)
    for qd in range(5):
        for a in range(2):
            for p in range(5):
                for b in range(2):
                    perm[qd * 20 + a * 10 + p * 2 + b] = \
                        20 * p + 16 - 4 * qd + 2 * a + b
    weT2 = np.ascontiguousarray(
        we[perm].transpose(2, 1, 0)).astype(np.float32)     # [9, 64, 100]
    weP = np.empty((3, 2 * CC, CM), np.float32)
    weS = np.empty((3, CC, CM), np.float32)
    for dy in range(3):
        weP[dy, 0:CC] = weT2[dy * 3 + 0]
        weP[dy, CC:2 * CC] = weT2[dy * 3 + 1]
        weS[dy] = weT2[dy * 3 + 2]
    weP = weP.astype(bfloat16)
    weS = weS.astype(bfloat16)
    benc2 = np.ascontiguousarray(
        np.asarray(b_enc, np.float32)[perm].reshape(CM, 1))
    ident = np.eye(CM, dtype=np.float32).astype(bfloat16)
    # stagger matrices: shf[k, qd*128 + j] = 1 iff k = j + qd - 2, same half
    shf = np.zeros((128, K5 * 128), np.float32)
    for qd in range(K5):
        for j in range(128):
            k = j + qd - 2
            if 0 <= k < 128 and k // 64 == j // 64:
                shf[k, qd * 128 + j] = 1.0
    shf = shf.astype(bfloat16)

    in_maps = []
    for core in range(NCORES):
        n, half = core // 2, core % 2
        h0 = RH * half
        slab = np.zeros((C, SLAB, W), np.float32)
        r_lo, r_hi = h0 - 2, h0 + SLAB - 2
        v_lo, v_hi = max(0, r_lo), min(H, r_hi)
        slab[:, v_lo - r_lo:v_hi - r_lo, :] = x[n, :, v_lo:v_hi, :]
        xs_b = slab.reshape(C, SLAB * W).astype(bfloat16)
        # xb[w', s*256 + ch] = slab[ch, s, w']
        xb = np.ascontiguousarray(
            slab.transpose(2, 1, 0).reshape(W, SLAB * C)).astype(bfloat16)
        in_maps.append({"xs": xs_b, "xb": xb, "wcT": wcT, "bc": bc,
                        "weP": weP, "weS": weS, "benc": benc2, "shf": shf,
                        "ident": ident})
    return in_maps


def host_gather(results):
    out = np.empty((N_B, C, S * H, S * W), np.float32)
    for core in range(NCORES):
        n, half = core // 2, core % 2
        out[n, :, HO * half:HO * (half + 1), :] = results[core]["out"]
    return out


def kernel(x, w_comp, b_comp, w_enc, b_enc):
    from concourse.bass_utils import run_bass_kernel_spmd
    nc = _get_program()
    in_maps = host_prep(x, w_comp, b_comp, w_enc, b_enc)
    res = run_bass_kernel_spmd(nc, in_maps, list(range(NCORES)))
    return host_gather(res.results)


# revision 61
# speedup vs baseline: 1.0744x; 1.0744x over previous
"""CARAFE (content-aware upsampling) Trainium2 kernel — v2.

Full inputs -> shard over 8 NeuronCores (batch x image-half) -> bass/Tile
kernel per core -> gather full output.

Reference semantics:
  comp = conv1x1(x, w_comp) + b_comp                    [n,64,64,64]
  mask = conv3x3(comp, w_enc, pad=1) + b_enc            [n,100,64,64]
  m    = softmax over 25 of pixel_shuffle(mask, 2)      [n,25,128,128]
  out[n,c,i,j] = sum_k m[n,k,i,j] * xpad[n,c,i//2+p, j//2+q],  k=5p+q

v2 design:
  - bf16 operands on-chip (fp32 PSUM accumulate, fp32 output)
  - mask conv channel-major (weights stationary), exp+bias fused in evac,
    PE transpose to pixel-major, DVE Z-reduce/recip/normalize
  - jl->w' stagger via 5 constant 0/1 shift-matrix matmuls (paired over
    2-block superblocks)
  - band shear via DRAM flat-address scatter (SBUF-side shears silently
    fail on HW) into reused, once-zeroed DRAM buffers; read back as
    [64, 4*HB2] with 40-col guards absorbing all clipped/invalid cells
  - x loaded once column-major [w', slabrow, ch] for carafe lhsT slices
"""
import numpy as np
import sys
from contextlib import ExitStack

sys.path.insert(0, "/opt/trn_rl_repo")

# ---------------- problem constants (hardcoded per spec) ----------------
N_B, C, H, W = 4, 256, 64, 64
CC = 64            # compressed channels
K5 = 5             # carafe kernel
S = 2              # scale
CM = K5 * K5 * S * S   # 100 mask channels
NCORES = 8
RH = H // 2        # 32 low-res rows per core
SLAB = RH + 4      # 36 x-rows per core (h0-2 .. h0+33)
NBLK = RH // 2     # 16 h-pair blocks
NSUP = NBLK // 2   # 8 superblocks (2 h-pair blocks each)
HO, WO = 2 * RH, 2 * W   # 64 x 128 output shard
GUARD = 40
HB2 = W * 20 + 2 * GUARD   # 1360 band cols per (rr)
SBW = 4 * HB2      # 5440 band cols per superblock (tt, rr)
COMP_W = W + 2     # 66 comp cols (1 zero col each side)
COMP_R = RH + 2    # 34 comp rows


def _build_program():
    import concourse.bass as bass
    import concourse.tile as tile
    from concourse import bacc, mybir
    from concourse.ap import AP
    AF = mybir.ActivationFunctionType

    def pstep(t):
        return t[:].ap[0][0]

    f32 = mybir.dt.float32
    bf16 = mybir.dt.bfloat16

    nc = bacc.Bacc("TRN2", target_bir_lowering=False, debug=False,
                   num_devices=NCORES)

    # ---------------- DRAM parameters ----------------
    xs = nc.dram_tensor("xs", [C, SLAB * W], bf16, kind="ExternalInput")
    xb = nc.dram_tensor("xb", [W, SLAB * C], bf16, kind="ExternalInput")
    wcT = nc.dram_tensor("wcT", [C, CC], bf16, kind="ExternalInput")
    bc = nc.dram_tensor("bc", [CC, 1], f32, kind="ExternalInput")
    weP = nc.dram_tensor("weP", [3, 2 * CC, CM], bf16, kind="ExternalInput")
    weS = nc.dram_tensor("weS", [3, CC, CM], bf16, kind="ExternalInput")
    benc = nc.dram_tensor("benc", [CM, 1], f32, kind="ExternalInput")
    shf = nc.dram_tensor("shf", [128, K5 * 128], bf16, kind="ExternalInput")
    ident = nc.dram_tensor("ident", [CM, CM], bf16, kind="ExternalInput")
    bndz = [nc.dram_tensor(f"bndz{i}", [W, SBW], bf16) for i in range(2)]
    out = nc.dram_tensor("out", [C, HO, WO], f32, kind="ExternalOutput")

    with tile.TileContext(nc) as tc:
        with ExitStack() as ctx:
            cpool = ctx.enter_context(tc.tile_pool(name="const", bufs=1))
            xpool = ctx.enter_context(tc.tile_pool(name="xdata", bufs=1))
            work = ctx.enter_context(tc.tile_pool(name="work", bufs=3))
            bpool = ctx.enter_context(tc.tile_pool(name="bandp", bufs=2))
            opool = ctx.enter_context(tc.tile_pool(name="oevac", bufs=3))
            ps_c_p = ctx.enter_context(tc.tile_pool(name="psc", bufs=1, space="PSUM"))
            ps_cm_p = ctx.enter_context(tc.tile_pool(name="pscm", bufs=2, space="PSUM"))
            ps_m_p = ctx.enter_context(tc.tile_pool(name="psm", bufs=2, space="PSUM"))
            ps_r_p = ctx.enter_context(tc.tile_pool(name="psr", bufs=1, space="PSUM"))
            ps_o_p = ctx.enter_context(tc.tile_pool(name="pso", bufs=2, space="PSUM"))

            # ---------------- load constants ----------------
            t_wc = [cpool.tile([128, CC], bf16, tag=f"wc{k}", name=f"wc{k}")
                    for k in range(2)]
            for k in range(2):
                nc.sync.dma_start(t_wc[k][:], wcT.ap()[128 * k:128 * (k + 1), :])
            t_bc = cpool.tile([CC, 1], f32, tag="bc", name="bc")
            nc.sync.dma_start(t_bc[:], bc.ap())
            t_weP = cpool.tile([2 * CC, 3 * CM], bf16, tag="weP", name="weP")
            src_wp = AP(weP.ap().tensor, 0,
                        [[CM, 2 * CC], [2 * CC * CM, 3], [1, CM]])
            nc.sync.dma_start(t_weP[:], src_wp)
            t_weS = cpool.tile([CC, 3 * CM], bf16, tag="weS", name="weS")
            src_ws = AP(weS.ap().tensor, 0,
                        [[CM, CC], [CC * CM, 3], [1, CM]])
            nc.sync.dma_start(t_weS[:], src_ws)
            t_benc = cpool.tile([CM, 1], f32, tag="benc", name="benc")
            nc.sync.dma_start(t_benc[:], benc.ap())
            t_id = cpool.tile([CM, CM], bf16, tag="ident", name="ident")
            nc.sync.dma_start(t_id[:], ident.ap())
            t_shf = cpool.tile([128, K5 * 128], bf16, tag="shf", name="shf")
            nc.sync.dma_start(t_shf[:], shf.ap())

            # ---------------- load x ----------------
            t_xs = [xpool.tile([128, SLAB * W], bf16, tag=f"xs{k}", name=f"xs{k}")
                    for k in range(2)]
            XSPL = 24 * W
            for k in range(2):
                nc.sync.dma_start(t_xs[k][:, 0:XSPL],
                                  xs.ap()[128 * k:128 * (k + 1), 0:XSPL])
            t_xb = xpool.tile([W, SLAB * C], bf16, tag="xb", name="xb")
            xbstep = pstep(t_xb)

            # ---------------- zero the DRAM band buffers (once) -------------
            # bndz[0] is needed by scatter(0) early; bndz[1]'s zero-write is
            # deferred below to keep it off the startup-critical DMA rings
            t_bz = bpool.tile([W, SBW], bf16, tag="band", name="bz")
            nc.gpsimd.memset(t_bz[:], 0.0)
            nc.scalar.dma_start(bndz[0].ap(), t_bz[:])

            # ------------- per superblock (2 h-pair blocks) -----------------
            # Software-pipelined: mask_phase(u+1) is emitted between
            # scatter(u) and carafe(u) so the PE has work while the band
            # round-trips through DRAM.
            def phase_A(u):
                """Mask math for superblock u: comp conv, 3x3 conv, exp,
                transpose, Z, reciprocal, normalize. Returns t_ms."""
                # comp = 1x1 conv + bias for this superblock's 6 rows
                # (comp rows 4u..4u+5 = slab rows 4u+1..4u+6); half 1 of the
                # partition dim holds comp shifted left one col for tap pairs
                t_comp = work.tile([2 * CC, 6 * COMP_W], bf16, tag="comp",
                                   name="comp")
                compv = t_comp[:].rearrange("p (r w) -> p r w", w=COMP_W)
                nc.vector.memset(compv[0:CC, :, 0:1], 0.0)
                nc.vector.memset(compv[0:CC, :, COMP_W - 1:COMP_W], 0.0)
                nc.vector.memset(compv[CC:2 * CC, :, COMP_W - 2:COMP_W], 0.0)
                cstep = pstep(t_comp)
                ps_c = ps_c_p.tile([CC, 6 * W], f32, tag="ps_c", name="ps_c")
                for k in range(2):
                    rhs = AP(t_xs[k][:].tensor, (4 * u + 1) * W,
                             [[pstep(t_xs[k]), 128], [1, 6 * W]])
                    nc.tensor.matmul(ps_c[:], t_wc[k][:], rhs,
                                     start=(k == 0), stop=(k == 1))
                psv = ps_c[:].rearrange("p (r w) -> p r w", w=W)
                nc.scalar.activation(compv[0:CC, :, 1:1 + W], psv,
                                     func=AF.Identity, bias=t_bc[:])
                nc.scalar.activation(compv[CC:2 * CC, :, 0:W], psv,
                                     func=AF.Identity, bias=t_bc[:])

                ps_m = ps_m_p.tile([128, 2 * CM], bf16, tag="ps_m", name="ps_m")
                mstep = ps_m[:].ap[0][0]
                t_z = work.tile([128, 8], f32, tag="z", name="z")
                zst = pstep(t_z)
                # mask conv, channel-major: ps_cm [100, 128 pix=(rr,w')]
                # tap pairs (dx=0,1) over 128 partitions + single dx=2;
                # the two tt interleave so consecutive matmuls hit
                # different PSUM tiles (accumulation chains pipeline)
                ps_cm = [ps_cm_p.tile([CM, 128], f32, tag="ps_cm",
                                      name=f"ps_cm{tt}") for tt in range(2)]
                for dy in range(3):
                    for tt in range(2):
                        rhsP = AP(t_comp[:].tensor, (2 * tt + dy) * COMP_W,
                                  [[cstep, 2 * CC], [COMP_W, 2], [1, W]])
                        nc.tensor.matmul(ps_cm[tt][:],
                                         t_weP[:, dy * CM:(dy + 1) * CM],
                                         rhsP, start=(dy == 0), stop=False)
                    for tt in range(2):
                        rhsS = AP(t_comp[:].tensor,
                                  (2 * tt + dy) * COMP_W + 2,
                                  [[cstep, CC], [COMP_W, 2], [1, W]])
                        nc.tensor.matmul(ps_cm[tt][:],
                                         t_weS[:, dy * CM:(dy + 1) * CM],
                                         rhsS, start=False, stop=(dy == 2))
                t_em = [None, None]
                for tt in range(2):
                    # exp(mask + b_enc) -> bf16, channel-major
                    t_em[tt] = work.tile([CM, 128], bf16, tag="em", name="em")
                    nc.scalar.activation(t_em[tt][:], ps_cm[tt][:],
                                         func=AF.Exp, bias=t_benc[:])
                for tt in range(2):
                    # transpose -> pixel-major psum segment [128, 100]
                    nc.tensor.transpose(ps_m[:, tt * CM:(tt + 1) * CM],
                                        t_em[tt][:], t_id[:])
                for tt in range(2):
                    # Z = sum over (qd, p) per (a, b)
                    in_z = AP(ps_m[:].tensor, tt * CM,
                              [[mstep, 128], [10, 2], [1, 2], [20, 5], [2, 5]])
                    oz = AP(t_z[:].tensor, 4 * tt,
                            [[zst, 128], [2, 2], [1, 2]])
                    nc.vector.tensor_reduce(oz, in_z,
                                            axis=mybir.AxisListType.XY,
                                            op=mybir.AluOpType.add)
                t_rz = work.tile([128, 8], bf16, tag="rz", name="rz")
                with nc.allow_low_precision(reason="1/Z well-conditioned"):
                    nc.vector.reciprocal(t_rz[:], t_z[:])
                zstep = pstep(t_rz)

                # normalize: t_ms = exp * rz; cols (qd, tt, a, p, b)
                t_ms = work.tile([128, 2 * CM], bf16, tag="ms", name="ms")
                sstep = pstep(t_ms)
                for tt in range(2):
                    for a in range(2):
                        for b in range(2):
                            o = AP(t_ms[:].tensor, tt * 20 + 10 * a + b,
                                   [[sstep, 128], [40, 5], [2, 5]])
                            i0 = AP(ps_m[:].tensor, tt * CM + 10 * a + b,
                                    [[mstep, 128], [20, 5], [2, 5]])
                            i1 = AP(t_rz[:].tensor, 4 * tt + 2 * a + b,
                                    [[zstep, 128], [0, 5], [0, 5]])
                            nc.vector.tensor_mul(o, i0, i1)
                return t_ms

            def phase_B(u, t_ms):
                """Stagger matmuls, evac, DRAM shear scatter, band read."""
                sstep = pstep(t_ms)
                # stagger via 5 shift-matmuls over both tt at once
                ps_r = ps_r_p.tile([128, 2 * CM], f32, tag="ps_r", name="ps_r")
                rpst = ps_r[:].ap[0][0]
                for qd in range(K5):
                    nc.tensor.matmul(ps_r[:, qd * 40:(qd + 1) * 40],
                                     t_shf[:, qd * 128:(qd + 1) * 128],
                                     t_ms[:, qd * 40:(qd + 1) * 40],
                                     start=True, stop=True)
                # evac halves -> t_rp2 [64, (tt, rr, qd*20+apb)]
                t_rp2 = work.tile([W, 4 * CM], bf16, tag="rp2", name="rp2")
                rstep = pstep(t_rp2)
                for rr in range(2):
                    o = AP(t_rp2[:].tensor, rr * CM,
                           [[rstep, W], [20, 5], [2 * CM, 2], [1, 20]])
                    i = AP(ps_r[:].tensor, 64 * rr * rpst,
                           [[rpst, W], [40, 5], [20, 2], [1, 20]])
                    if rr == 0:
                        nc.vector.tensor_copy(o, i)
                    else:
                        nc.scalar.activation(o, i, func=AF.Copy)

                # sheared scatter SBUF -> DRAM (flat), one DMA per tt
                for tt in range(2):
                    dst = AP(bndz[u % 2].ap().tensor, tt * 2 * HB2,
                             [[SBW + 20, W], [HB2, 2], [1, CM]])
                    src = AP(t_rp2[:].tensor, tt * 2 * CM,
                             [[rstep, W], [CM, 2], [1, CM]])
                    with nc.allow_non_contiguous_dma(reason="band shear"):
                        nc.sync.dma_start(dst, src)
                # read back the full superblock band
                bnd = bpool.tile([W, SBW], bf16, tag="band", name="band")
                nc.scalar.dma_start(bnd[:], bndz[u % 2].ap())
                return bnd

            def carafe_phase(u, bnd):
                bstep = pstep(bnd)
                t_o = [opool.tile([128, 1024], f32, tag=f"osb{ct}",
                                  name=f"osb{ct}") for ct in range(2)]
                for tt in range(2):
                    t = 2 * u + tt
                    for ct in range(2):
                        pso = ps_o_p.tile([128, 512], f32, tag="ps_o",
                                          name="ps_o")
                        for rr in range(2):
                            for p in range(K5):
                                lhsT = AP(t_xb[:].tensor,
                                          (2 * t + p + rr) * C + ct * 128,
                                          [[xbstep, W], [1, 128]])
                                rhs = AP(bnd[:].tensor,
                                         tt * 2 * HB2 + rr * HB2 + GUARD + 2 * p,
                                         [[bstep, W], [10, 2], [20, W], [1, 2]])
                                nc.tensor.matmul(
                                    pso[:, rr * 256:(rr + 1) * 256],
                                    lhsT, rhs, start=(p == 0),
                                    stop=(p == K5 - 1))
                        dst = t_o[ct][:, tt * 512:(tt + 1) * 512]
                        if ct == 0:
                            nc.vector.tensor_copy(dst, pso[:])
                        else:
                            nc.scalar.activation(dst, pso[:], func=AF.Copy)
                for ct in range(2):
                    for tt in range(2):
                        dsto = AP(out.ap().tensor,
                                  ct * 128 * HO * WO + (8 * u + 4 * tt) * WO,
                                  [[HO * WO, 128], [1, 512]])
                        nc.sync.dma_start(dsto,
                                          t_o[ct][:, tt * 512:(tt + 1) * 512])

            # 3-stage software pipeline: the PE stream per iteration is
            # stagger(u) -> convs/transposes(u+2) -> carafe(u-1), with each
            # group's dependencies produced at least one iteration earlier.
            tms = {0: phase_A(0)}
            nc.scalar.dma_start(bndz[1].ap(), t_bz[:])
            for k in range(2):
                nc.scalar.dma_start(t_xs[k][:, XSPL:SLAB * W],
                                    xs.ap()[128 * k:128 * (k + 1),
                                            XSPL:SLAB * W])
            tms[1] = phase_A(1)
            bnds = {}
            for u in range(NSUP):
                bnds[u] = phase_B(u, tms.pop(u))
                if u == 0:
                    # xb is first needed by carafe(0); defer its load behind
                    # the startup-critical xs/zero/scatter/read DMAs
                    nc.sync.dma_start(t_xb[:], xb.ap())
                if u + 2 < NSUP:
                    tms[u + 2] = phase_A(u + 2)
                if u >= 1:
                    carafe_phase(u - 1, bnds.pop(u - 1))
            carafe_phase(NSUP - 1, bnds.pop(NSUP - 1))

    nc.compile()
    return nc


_CACHE = {}


def _get_program():
    if "nc" not in _CACHE:
        _CACHE["nc"] = _build_program()
    return _CACHE["nc"]


def host_prep(x, w_comp, b_comp, w_enc, b_enc):
    """Build per-core input maps."""
    from ml_dtypes import bfloat16
    x = np.asarray(x, dtype=np.float32)
    wcT = np.ascontiguousarray(
        np.asarray(w_comp, np.float32).reshape(CC, C).T).astype(bfloat16)
    bc = np.asarray(b_comp, np.float32).reshape(CC, 1)
    # weT2[k, cin, qd*20 + a*10 + p*2 + b] = w_enc[20p+16-4qd+2a+b, cin, dy, dx]
    we = np.asarray(w_enc, np.float32).reshape(CM, CC, 9)
    perm = np.empty(CM, np.int64)
    for qd in range(5):
        for a in range(2):
            for p in range(5):
                for b in range(2):
                    perm[qd * 20 + a * 10 + p * 2 + b] = \
                        20 * p + 16 - 4 * qd + 2 * a + b
    weT2 = np.ascontiguousarray(
        we[perm].transpose(2, 1, 0)).astype(np.float32)     # [9, 64, 100]
    weP = np.empty((3, 2 * CC, CM), np.float32)
    weS = np.empty((3, CC, CM), np.float32)
    for dy in range(3):
        weP[dy, 0:CC] = weT2[dy * 3 + 0]
        weP[dy, CC:2 * CC] = weT2[dy * 3 + 1]
        weS[dy] = weT2[dy * 3 + 2]
    weP = weP.astype(bfloat16)
    weS = weS.astype(bfloat16)
    benc2 = np.ascontiguousarray(
        np.asarray(b_enc, np.float32)[perm].reshape(CM, 1))
    ident = np.eye(CM, dtype=np.float32).astype(bfloat16)
    # stagger matrices: shf[k, qd*128 + j] = 1 iff k = j + qd - 2, same half
    shf = np.zeros((128, K5 * 128), np.float32)
    for qd in range(K5):
        for j in range(128):
            k = j + qd - 2
            if 0 <= k < 128 and k // 64 == j // 64:
                shf[k, qd * 128 + j] = 1.0
    shf = shf.astype(bfloat16)

    in_maps = []
    for core in range(NCORES):
        n, half = core // 2, core % 2
        h0 = RH * half
        slab = np.zeros((C, SLAB, W), np.float32)
        r_lo, r_hi = h0 - 2, h0 + SLAB - 2
        v_lo, v_hi = max(0, r_lo), min(H, r_hi)
        slab[:, v_lo - r_lo:v_hi - r_lo, :] = x[n, :, v_lo:v_hi, :]
        xs_b = slab.reshape(C, SLAB * W).astype(bfloat16)
        # xb[w', s*256 + ch] = slab[ch, s, w']
        xb = np.ascontiguousarray(
            slab.transpose(2, 1, 0).reshape(W, SLAB * C)).astype(bfloat16)
        in_maps.append({"xs": xs_b, "xb": xb, "wcT": wcT, "bc": bc,
                        "weP": weP, "weS": weS, "benc": benc2, "shf": shf,
                        "ident": ident})
    return in_maps


def host_gather(results):
    out = np.empty((N_B, C, S * H, S * W), np.float32)
    for core in range(NCORES):
        n, half = core // 2, core % 2
        out[n, :, HO * half:HO * (half + 1), :] = results[core]["out"]
    return out


def kernel(x, w_comp, b_comp, w_enc, b_enc):
    from concourse.bass_utils import run_bass_kernel_spmd
    nc = _get_program()
    in_maps = host_prep(x, w_comp, b_comp, w_enc, b_enc)
    res = run_bass_kernel_spmd(nc, in_maps, list(range(NCORES)))
    return host_gather(res.results)


# revision 62
# speedup vs baseline: 1.0932x; 1.0175x over previous
"""CARAFE (content-aware upsampling) Trainium2 kernel — v2.

Full inputs -> shard over 8 NeuronCores (batch x image-half) -> bass/Tile
kernel per core -> gather full output.

Reference semantics:
  comp = conv1x1(x, w_comp) + b_comp                    [n,64,64,64]
  mask = conv3x3(comp, w_enc, pad=1) + b_enc            [n,100,64,64]
  m    = softmax over 25 of pixel_shuffle(mask, 2)      [n,25,128,128]
  out[n,c,i,j] = sum_k m[n,k,i,j] * xpad[n,c,i//2+p, j//2+q],  k=5p+q

v2 design:
  - bf16 operands on-chip (fp32 PSUM accumulate, fp32 output)
  - mask conv channel-major (weights stationary), exp+bias fused in evac,
    PE transpose to pixel-major, DVE Z-reduce/recip/normalize
  - jl->w' stagger via 5 constant 0/1 shift-matrix matmuls (paired over
    2-block superblocks)
  - band shear via DRAM flat-address scatter (SBUF-side shears silently
    fail on HW) into reused, once-zeroed DRAM buffers; read back as
    [64, 4*HB2] with 40-col guards absorbing all clipped/invalid cells
  - x loaded once column-major [w', slabrow, ch] for carafe lhsT slices
"""
import numpy as np
import sys
from contextlib import ExitStack

sys.path.insert(0, "/opt/trn_rl_repo")

# ---------------- problem constants (hardcoded per spec) ----------------
N_B, C, H, W = 4, 256, 64, 64
CC = 64            # compressed channels
K5 = 5             # carafe kernel
S = 2              # scale
CM = K5 * K5 * S * S   # 100 mask channels
NCORES = 8
RH = H // 2        # 32 low-res rows per core
SLAB = RH + 4      # 36 x-rows per core (h0-2 .. h0+33)
NBLK = RH // 2     # 16 h-pair blocks
NSUP = NBLK // 2   # 8 superblocks (2 h-pair blocks each)
HO, WO = 2 * RH, 2 * W   # 64 x 128 output shard
GUARD = 40
HB2 = W * 20 + 2 * GUARD   # 1360 band cols per (rr)
SBW = 4 * HB2      # 5440 band cols per superblock (tt, rr)
COMP_W = W + 2     # 66 comp cols (1 zero col each side)
COMP_R = RH + 2    # 34 comp rows


def _build_program():
    import concourse.bass as bass
    import concourse.tile as tile
    from concourse import bacc, mybir
    from concourse.ap import AP
    AF = mybir.ActivationFunctionType

    def pstep(t):
        return t[:].ap[0][0]

    f32 = mybir.dt.float32
    bf16 = mybir.dt.bfloat16

    nc = bacc.Bacc("TRN2", target_bir_lowering=False, debug=False,
                   num_devices=NCORES)

    # ---------------- DRAM parameters ----------------
    xs = nc.dram_tensor("xs", [C, SLAB * W], bf16, kind="ExternalInput")
    xb = nc.dram_tensor("xb", [W, SLAB * C], bf16, kind="ExternalInput")
    wcT = nc.dram_tensor("wcT", [C, CC], bf16, kind="ExternalInput")
    bc = nc.dram_tensor("bc", [CC, 1], f32, kind="ExternalInput")
    weP = nc.dram_tensor("weP", [3, 2 * CC, CM], bf16, kind="ExternalInput")
    weS = nc.dram_tensor("weS", [3, CC, CM], bf16, kind="ExternalInput")
    benc = nc.dram_tensor("benc", [CM, 1], f32, kind="ExternalInput")
    shf = nc.dram_tensor("shf", [128, K5 * 128], bf16, kind="ExternalInput")
    ident = nc.dram_tensor("ident", [CM, CM], bf16, kind="ExternalInput")
    bndz = [nc.dram_tensor(f"bndz{i}", [W, SBW], bf16) for i in range(2)]
    out = nc.dram_tensor("out", [C, HO, WO], f32, kind="ExternalOutput")

    with tile.TileContext(nc) as tc:
        with ExitStack() as ctx:
            cpool = ctx.enter_context(tc.tile_pool(name="const", bufs=1))
            xpool = ctx.enter_context(tc.tile_pool(name="xdata", bufs=1))
            work = ctx.enter_context(tc.tile_pool(name="work", bufs=4))
            bpool = ctx.enter_context(tc.tile_pool(name="bandp", bufs=3))
            opool = ctx.enter_context(tc.tile_pool(name="oevac", bufs=4))
            ps_c_p = ctx.enter_context(tc.tile_pool(name="psc", bufs=1, space="PSUM"))
            ps_cm_p = ctx.enter_context(tc.tile_pool(name="pscm", bufs=2, space="PSUM"))
            ps_m_p = ctx.enter_context(tc.tile_pool(name="psm", bufs=2, space="PSUM"))
            ps_r_p = ctx.enter_context(tc.tile_pool(name="psr", bufs=1, space="PSUM"))
            ps_o_p = ctx.enter_context(tc.tile_pool(name="pso", bufs=2, space="PSUM"))

            # ---------------- load constants ----------------
            t_wc = [cpool.tile([128, CC], bf16, tag=f"wc{k}", name=f"wc{k}")
                    for k in range(2)]
            for k in range(2):
                nc.sync.dma_start(t_wc[k][:], wcT.ap()[128 * k:128 * (k + 1), :])
            t_bc = cpool.tile([CC, 1], f32, tag="bc", name="bc")
            nc.sync.dma_start(t_bc[:], bc.ap())
            t_weP = cpool.tile([2 * CC, 3 * CM], bf16, tag="weP", name="weP")
            src_wp = AP(weP.ap().tensor, 0,
                        [[CM, 2 * CC], [2 * CC * CM, 3], [1, CM]])
            nc.sync.dma_start(t_weP[:], src_wp)
            t_weS = cpool.tile([CC, 3 * CM], bf16, tag="weS", name="weS")
            src_ws = AP(weS.ap().tensor, 0,
                        [[CM, CC], [CC * CM, 3], [1, CM]])
            nc.sync.dma_start(t_weS[:], src_ws)
            t_benc = cpool.tile([CM, 1], f32, tag="benc", name="benc")
            nc.sync.dma_start(t_benc[:], benc.ap())
            t_id = cpool.tile([CM, CM], bf16, tag="ident", name="ident")
            nc.sync.dma_start(t_id[:], ident.ap())
            t_shf = cpool.tile([128, K5 * 128], bf16, tag="shf", name="shf")
            nc.sync.dma_start(t_shf[:], shf.ap())

            # ---------------- load x ----------------
            t_xs = [xpool.tile([128, SLAB * W], bf16, tag=f"xs{k}", name=f"xs{k}")
                    for k in range(2)]
            XSPL = 24 * W
            for k in range(2):
                nc.sync.dma_start(t_xs[k][:, 0:XSPL],
                                  xs.ap()[128 * k:128 * (k + 1), 0:XSPL])
            t_xb = xpool.tile([W, SLAB * C], bf16, tag="xb", name="xb")
            xbstep = pstep(t_xb)

            # ---------------- zero the DRAM band buffers (once) -------------
            # bndz[0] is needed by scatter(0) early; bndz[1]'s zero-write is
            # deferred below to keep it off the startup-critical DMA rings
            t_bz = bpool.tile([W, SBW], bf16, tag="band", name="bz")
            nc.gpsimd.memset(t_bz[:], 0.0)
            nc.scalar.dma_start(bndz[0].ap(), t_bz[:])

            # ------------- per superblock (2 h-pair blocks) -----------------
            # Software-pipelined: mask_phase(u+1) is emitted between
            # scatter(u) and carafe(u) so the PE has work while the band
            # round-trips through DRAM.
            def phase_A(u):
                """Mask math for superblock u: comp conv, 3x3 conv, exp,
                transpose, Z, reciprocal, normalize. Returns t_ms."""
                # comp = 1x1 conv + bias for this superblock's 6 rows
                # (comp rows 4u..4u+5 = slab rows 4u+1..4u+6); half 1 of the
                # partition dim holds comp shifted left one col for tap pairs
                t_comp = work.tile([2 * CC, 6 * COMP_W], bf16, tag="comp",
                                   name="comp")
                compv = t_comp[:].rearrange("p (r w) -> p r w", w=COMP_W)
                nc.vector.memset(compv[0:CC, :, 0:1], 0.0)
                nc.vector.memset(compv[0:CC, :, COMP_W - 1:COMP_W], 0.0)
                nc.vector.memset(compv[CC:2 * CC, :, COMP_W - 2:COMP_W], 0.0)
                cstep = pstep(t_comp)
                ps_c = ps_c_p.tile([CC, 6 * W], f32, tag="ps_c", name="ps_c")
                for k in range(2):
                    rhs = AP(t_xs[k][:].tensor, (4 * u + 1) * W,
                             [[pstep(t_xs[k]), 128], [1, 6 * W]])
                    nc.tensor.matmul(ps_c[:], t_wc[k][:], rhs,
                                     start=(k == 0), stop=(k == 1))
                psv = ps_c[:].rearrange("p (r w) -> p r w", w=W)
                nc.scalar.activation(compv[0:CC, :, 1:1 + W], psv,
                                     func=AF.Identity, bias=t_bc[:])
                nc.scalar.activation(compv[CC:2 * CC, :, 0:W], psv,
                                     func=AF.Identity, bias=t_bc[:])

                ps_m = ps_m_p.tile([128, 2 * CM], bf16, tag="ps_m", name="ps_m")
                mstep = ps_m[:].ap[0][0]
                t_z = work.tile([128, 8], f32, tag="z", name="z")
                zst = pstep(t_z)
                # mask conv, channel-major: ps_cm [100, 128 pix=(rr,w')]
                # tap pairs (dx=0,1) over 128 partitions + single dx=2;
                # the two tt interleave so consecutive matmuls hit
                # different PSUM tiles (accumulation chains pipeline)
                ps_cm = [ps_cm_p.tile([CM, 128], f32, tag="ps_cm",
                                      name=f"ps_cm{tt}") for tt in range(2)]
                for dy in range(3):
                    for tt in range(2):
                        rhsP = AP(t_comp[:].tensor, (2 * tt + dy) * COMP_W,
                                  [[cstep, 2 * CC], [COMP_W, 2], [1, W]])
                        nc.tensor.matmul(ps_cm[tt][:],
                                         t_weP[:, dy * CM:(dy + 1) * CM],
                                         rhsP, start=(dy == 0), stop=False)
                    for tt in range(2):
                        rhsS = AP(t_comp[:].tensor,
                                  (2 * tt + dy) * COMP_W + 2,
                                  [[cstep, CC], [COMP_W, 2], [1, W]])
                        nc.tensor.matmul(ps_cm[tt][:],
                                         t_weS[:, dy * CM:(dy + 1) * CM],
                                         rhsS, start=False, stop=(dy == 2))
                t_em = [None, None]
                for tt in range(2):
                    # exp(mask + b_enc) -> bf16, channel-major
                    t_em[tt] = work.tile([CM, 128], bf16, tag="em", name="em")
                    nc.scalar.activation(t_em[tt][:], ps_cm[tt][:],
                                         func=AF.Exp, bias=t_benc[:])
                for tt in range(2):
                    # transpose -> pixel-major psum segment [128, 100]
                    nc.tensor.transpose(ps_m[:, tt * CM:(tt + 1) * CM],
                                        t_em[tt][:], t_id[:])
                for tt in range(2):
                    # Z = sum over (qd, p) per (a, b)
                    in_z = AP(ps_m[:].tensor, tt * CM,
                              [[mstep, 128], [10, 2], [1, 2], [20, 5], [2, 5]])
                    oz = AP(t_z[:].tensor, 4 * tt,
                            [[zst, 128], [2, 2], [1, 2]])
                    nc.vector.tensor_reduce(oz, in_z,
                                            axis=mybir.AxisListType.XY,
                                            op=mybir.AluOpType.add)
                t_rz = work.tile([128, 8], bf16, tag="rz", name="rz")
                with nc.allow_low_precision(reason="1/Z well-conditioned"):
                    nc.vector.reciprocal(t_rz[:], t_z[:])
                zstep = pstep(t_rz)

                # normalize: t_ms = exp * rz; cols (qd, tt, a, p, b)
                t_ms = work.tile([128, 2 * CM], bf16, tag="ms", name="ms")
                sstep = pstep(t_ms)
                for tt in range(2):
                    for a in range(2):
                        for b in range(2):
                            o = AP(t_ms[:].tensor, tt * 20 + 10 * a + b,
                                   [[sstep, 128], [40, 5], [2, 5]])
                            i0 = AP(ps_m[:].tensor, tt * CM + 10 * a + b,
                                    [[mstep, 128], [20, 5], [2, 5]])
                            i1 = AP(t_rz[:].tensor, 4 * tt + 2 * a + b,
                                    [[zstep, 128], [0, 5], [0, 5]])
                            nc.vector.tensor_mul(o, i0, i1)
                return t_ms

            def phase_B(u, t_ms):
                """Stagger matmuls, evac, DRAM shear scatter, band read."""
                sstep = pstep(t_ms)
                # stagger via 5 shift-matmuls over both tt at once
                ps_r = ps_r_p.tile([128, 2 * CM], f32, tag="ps_r", name="ps_r")
                rpst = ps_r[:].ap[0][0]
                for qd in range(K5):
                    nc.tensor.matmul(ps_r[:, qd * 40:(qd + 1) * 40],
                                     t_shf[:, qd * 128:(qd + 1) * 128],
                                     t_ms[:, qd * 40:(qd + 1) * 40],
                                     start=True, stop=True)
                # evac halves -> t_rp2 [64, (tt, rr, qd*20+apb)]
                t_rp2 = work.tile([W, 4 * CM], bf16, tag="rp2", name="rp2")
                rstep = pstep(t_rp2)
                for rr in range(2):
                    o = AP(t_rp2[:].tensor, rr * CM,
                           [[rstep, W], [20, 5], [2 * CM, 2], [1, 20]])
                    i = AP(ps_r[:].tensor, 64 * rr * rpst,
                           [[rpst, W], [40, 5], [20, 2], [1, 20]])
                    if rr == 0:
                        nc.vector.tensor_copy(o, i)
                    else:
                        nc.scalar.activation(o, i, func=AF.Copy)

                # sheared scatter SBUF -> DRAM (flat), one DMA per tt
                for tt in range(2):
                    dst = AP(bndz[u % 2].ap().tensor, tt * 2 * HB2,
                             [[SBW + 20, W], [HB2, 2], [1, CM]])
                    src = AP(t_rp2[:].tensor, tt * 2 * CM,
                             [[rstep, W], [CM, 2], [1, CM]])
                    with nc.allow_non_contiguous_dma(reason="band shear"):
                        nc.sync.dma_start(dst, src)
                # read back the full superblock band
                bnd = bpool.tile([W, SBW], bf16, tag="band", name="band")
                nc.scalar.dma_start(bnd[:, 0:2 * HB2],
                                    bndz[u % 2].ap()[:, 0:2 * HB2])
                nc.sync.dma_start(bnd[:, 2 * HB2:SBW],
                                  bndz[u % 2].ap()[:, 2 * HB2:SBW])
                return bnd

            def carafe_phase(u, bnd):
                bstep = pstep(bnd)
                t_o = [opool.tile([128, 1024], f32, tag=f"osb{ct}",
                                  name=f"osb{ct}") for ct in range(2)]
                for tt in range(2):
                    t = 2 * u + tt
                    for ct in range(2):
                        pso = ps_o_p.tile([128, 512], f32, tag="ps_o",
                                          name="ps_o")
                        for rr in range(2):
                            for p in range(K5):
                                lhsT = AP(t_xb[:].tensor,
                                          (2 * t + p + rr) * C + ct * 128,
                                          [[xbstep, W], [1, 128]])
                                rhs = AP(bnd[:].tensor,
                                         tt * 2 * HB2 + rr * HB2 + GUARD + 2 * p,
                                         [[bstep, W], [10, 2], [20, W], [1, 2]])
                                nc.tensor.matmul(
                                    pso[:, rr * 256:(rr + 1) * 256],
                                    lhsT, rhs, start=(p == 0),
                                    stop=(p == K5 - 1))
                        dst = t_o[ct][:, tt * 512:(tt + 1) * 512]
                        if ct == 0:
                            nc.vector.tensor_copy(dst, pso[:])
                        else:
                            nc.scalar.activation(dst, pso[:], func=AF.Copy)
                for ct in range(2):
                    for tt in range(2):
                        dsto = AP(out.ap().tensor,
                                  ct * 128 * HO * WO + (8 * u + 4 * tt) * WO,
                                  [[HO * WO, 128], [1, 512]])
                        nc.sync.dma_start(dsto,
                                          t_o[ct][:, tt * 512:(tt + 1) * 512])

            # 3-stage software pipeline: the PE stream per iteration is
            # stagger(u) -> convs/transposes(u+2) -> carafe(u-1), with each
            # group's dependencies produced at least one iteration earlier.
            tms = {0: phase_A(0)}
            nc.scalar.dma_start(bndz[1].ap(), t_bz[:])
            for k in range(2):
                nc.scalar.dma_start(t_xs[k][:, XSPL:SLAB * W],
                                    xs.ap()[128 * k:128 * (k + 1),
                                            XSPL:SLAB * W])
            tms[1] = phase_A(1)
            bnds = {}
            for u in range(NSUP):
                bnds[u] = phase_B(u, tms.pop(u))
                if u == 0:
                    # xb is first needed by carafe(0); defer its load behind
                    # the startup-critical xs/zero/scatter/read DMAs
                    nc.sync.dma_start(t_xb[:], xb.ap())
                if u + 2 < NSUP:
                    tms[u + 2] = phase_A(u + 2)
                if u >= 1:
                    carafe_phase(u - 1, bnds.pop(u - 1))
            carafe_phase(NSUP - 1, bnds.pop(NSUP - 1))

    nc.compile()
    return nc


_CACHE = {}


def _get_program():
    if "nc" not in _CACHE:
        _CACHE["nc"] = _build_program()
    return _CACHE["nc"]


def host_prep(x, w_comp, b_comp, w_enc, b_enc):
    """Build per-core input maps."""
    from ml_dtypes import bfloat16
    x = np.asarray(x, dtype=np.float32)
    wcT = np.ascontiguousarray(
        np.asarray(w_comp, np.float32).reshape(CC, C).T).astype(bfloat16)
    bc = np.asarray(b_comp, np.float32).reshape(CC, 1)
    # weT2[k, cin, qd*20 + a*10 + p*2 + b] = w_enc[20p+16-4qd+2a+b, cin, dy, dx]
    we = np.asarray(w_enc, np.float32).reshape(CM, CC, 9)
    perm = np.empty(CM, np.int64)
    for qd in range(5):
        for a in range(2):
            for p in range(5):
                for b in range(2):
                    perm[qd * 20 + a * 10 + p * 2 + b] = \
                        20 * p + 16 - 4 * qd + 2 * a + b
    weT2 = np.ascontiguousarray(
        we[perm].transpose(2, 1, 0)).astype(np.float32)     # [9, 64, 100]
    weP = np.empty((3, 2 * CC, CM), np.float32)
    weS = np.empty((3, CC, CM), np.float32)
    for dy in range(3):
        weP[dy, 0:CC] = weT2[dy * 3 + 0]
        weP[dy, CC:2 * CC] = weT2[dy * 3 + 1]
        weS[dy] = weT2[dy * 3 + 2]
    weP = weP.astype(bfloat16)
    weS = weS.astype(bfloat16)
    benc2 = np.ascontiguousarray(
        np.asarray(b_enc, np.float32)[perm].reshape(CM, 1))
    ident = np.eye(CM, dtype=np.float32).astype(bfloat16)
    # stagger matrices: shf[k, qd*128 + j] = 1 iff k = j + qd - 2, same half
    shf = np.zeros((128, K5 * 128), np.float32)
    for qd in range(K5):
        for j in range(128):
            k = j + qd - 2
            if 0 <= k < 128 and k // 64 == j // 64:
                shf[k, qd * 128 + j] = 1.0
    shf = shf.astype(bfloat16)

    in_maps = []
    for core in range(NCORES):
        n, half = core // 2, core % 2
        h0 = RH * half
        slab = np.zeros((C, SLAB, W), np.float32)
        r_lo, r_hi = h0 - 2, h0 + SLAB - 2
        v_lo, v_hi = max(0, r_lo), min(H, r_hi)
        slab[:, v_lo - r_lo:v_hi - r_lo, :] = x[n, :, v_lo:v_hi, :]
        xs_b = slab.reshape(C, SLAB * W).astype(bfloat16)
        # xb[w', s*256 + ch] = slab[ch, s, w']
        xb = np.ascontiguousarray(
            slab.transpose(2, 1, 0).reshape(W, SLAB * C)).astype(bfloat16)
        in_maps.append({"xs": xs_b, "xb": xb, "wcT": wcT, "bc": bc,
                        "weP": weP, "weS": weS, "benc": benc2, "shf": shf,
                        "ident": ident})
    return in_maps


def host_gather(results):
    out = np.empty((N_B, C, S * H, S * W), np.float32)
    for core in range(NCORES):
        n, half = core // 2, core % 2
        out[n, :, HO * half:HO * (half + 1), :] = results[core]["out"]
    return out


def kernel(x, w_comp, b_comp, w_enc, b_enc):
    from concourse.bass_utils import run_bass_kernel_spmd
    nc = _get_program()
    in_maps = host_prep(x, w_comp, b_comp, w_enc, b_enc)
    res = run_bass_kernel_spmd(nc, in_maps, list(range(NCORES)))
    return host_gather(res.results)
